# revision 1
# baseline (speedup 1.0000x reference)
"""GCN 2-layer + linear head on 8 Trainium2 NeuronCores (Bass/Tile).

Strategy (matches the sharding hint):
- Nodes sharded 8 x 12500; edges partitioned by destination shard so the
  segment-sum is core-local.
- Normalization folded node-wise: out = dinv * A_hat @ (dinv * (x@W)),
  so no per-edge norm is needed (dinv computed host-side from in-degrees).
- Each core computes hs = dinv*(x@W) for its nodes as an fp16 [12500,128]
  row-padded table (256B rows), AllGather -> full [100000,128] table.
- Aggregation: edges are bucketed per (dst-block of 112 nodes, src-chunk
  of 25000) and padded to 128-edge tiles; source rows are fetched with
  large dma_gather batches (int16 chunk-local indices); a one-hot
  selector S (is_equal vs an iota, built on-device) turns the
  segment-sum into PSUM-accumulated fp16 matmuls on the TensorEngine.
- Layer 2 reuses the same machinery; the classifier is a K=64 matmul
  with Wc as the stationary operand.

SPMD constraint: all 8 cores execute one identical program, so
per-(block,chunk) tile counts are padded to the max across cores.
"""

import numpy as np

import concourse.bacc as bacc
import concourse.mybir as mybir
import concourse.tile as tile
from concourse.bass_utils import run_bass_kernel_spmd

# problem shapes (hardcoded per contract)
N = 100000
E = 1600000
FIN = 128
HID = 64

NC_ = 8
P = 128
FP = 128                   # padded feature width (fp16 -> 256B gather rows)
NLOC = N // NC_            # 12500
NCHUNK = 4
CHUNK = N // NCHUNK        # 25000  (< 32768 so int16 indices work)
QS = NLOC // NCHUNK        # 3125 rows of each rank per chunk
BS = 112                   # nodes per aggregation block (dst_rel < 128)
NBLK = (NLOC + BS - 1) // BS   # 112
SBB = 8                    # blocks per superblock (gather/S granularity)
NSB = (NBLK + SBB - 1) // SBB  # 14
NROW = (NLOC + P - 1) // P     # 98 row-tiles for hs production

# knockout flags for cost-model attribution experiments (prof only)
SKIP_MM = False
SKIP_GATHER = False
SKIP_S = False


# ----------------------------------------------------------------- host prep
def _prep(x, edge_index):
    """Build per-core device inputs + the (core-uniform) tile-count grid."""
    x = np.asarray(x, np.float32)
    src_g = np.asarray(edge_index[0], np.int64)
    dst_g = np.asarray(edge_index[1], np.int64)

    deg = np.bincount(dst_g, minlength=N).astype(np.float32) + 1.0
    dinv = (1.0 / np.sqrt(deg)).astype(np.float32)

    cores = []
    counts_all = np.zeros((NC_, NBLK * NCHUNK), np.int64)
    # self-loops are NOT in the edge lists: each node's own contribution is
    # added directly in the epilogue (it is core-local), which also keeps
    # the per-(block,chunk) counts balanced across chunks.
    for k in range(NC_):
        m = (dst_g // NLOC) == k
        s = src_g[m]
        d = dst_g[m] - k * NLOC
        b = d // BS
        sk = s // NLOC            # owning core of the source
        sp_ = s % NLOC            # position within that core
        c = sp_ // QS             # quarter-of-rank = chunk
        il = (sk * QS + sp_ % QS).astype(np.int16)
        dr = (d % BS).astype(np.int64)
        key = b * NCHUNK + c
        o = np.argsort(key, kind="stable")
        counts_all[k] = np.bincount(key, minlength=NBLK * NCHUNK)
        cores.append((il[o], dr[o], np.concatenate([[0], np.cumsum(counts_all[k])])))

    nt = ((counts_all.max(axis=0) + P - 1) // P).astype(np.int64)
    nt = np.maximum(nt, 1).reshape(NBLK, NCHUNK)

    seg_tiles = np.array([[nt[sb * SBB:min((sb + 1) * SBB, NBLK), c].sum()
                           for c in range(NCHUNK)] for sb in range(NSB)])
    nt_max = int(seg_tiles.max())
    tt = int(seg_tiles.sum())          # total tiles per layer

    in_maps = []
    for k in range(NC_):
        il_s, dr_s, cum = cores[k]
        idx_cols, dst_cols = [], []
        for sb in range(NSB):
            for c in range(NCHUNK):
                ils, drs = [], []
                for b in range(sb * SBB, min((sb + 1) * SBB, NBLK)):
                    g = b * NCHUNK + c
                    a0, a1 = cum[g], cum[g + 1]
                    npad = int(nt[b, c] * P - (a1 - a0))
                    ils.append(il_s[a0:a1])
                    ils.append(np.zeros(npad, np.int16))
                    drs.append(dr_s[a0:a1])
                    drs.append(np.full(npad, 200, np.int64))
                seg_il = np.concatenate(ils)
                seg_dr = np.concatenate(drs)
                wrapped = seg_il.reshape(-1, 16).T          # [16, n/16]
                idx_cols.append(np.tile(wrapped, (8, 1)))   # [128, n/16]
                dst_cols.append(seg_dr.astype(np.float16).reshape(-1, P).T)

        xT = np.zeros((FIN, NROW * P), np.float32)
        xT[:, :NLOC] = x[k * NLOC:(k + 1) * NLOC].T
        dloc = dinv[k * NLOC:(k + 1) * NLOC]
        dpad = np.zeros(NROW * P, np.float32)       # by 128-row tiles
        dpad[:NLOC] = dloc
        dcol1 = dpad.reshape(NROW, P).T.copy()
        dcol2 = np.zeros((P, NBLK), np.float32)     # by BS-node blocks
        for b in range(NBLK):
            w = min(BS, NLOC - b * BS)
            dcol2[:w, b] = dloc[b * BS:b * BS + w]
        wnb = max(NBLK * BS, NROW * P)
        dbc = np.zeros((HID, wnb), np.float32)
        dbc[:, :NLOC] = np.broadcast_to(dloc, (HID, NLOC))

        in_maps.append({
            "xT": xT,
            "idx": np.concatenate(idx_cols, axis=1),        # [128, tt*8] i16
            "dstrel": np.concatenate(dst_cols, axis=1),     # [128, tt] fp16
            "iota": np.tile(np.arange(P, dtype=np.float16), (P, nt_max)),
            "dinv_col1": dcol1,
            "dinv_col2": dcol2,
            "dinv_bc": dbc,                                 # [64, NBLK*BS]
        })
    return in_maps, nt, seg_tiles, nt_max, tt


# ------------------------------------------------------------- device build
def _build(nt, seg_tiles, nt_max, tt):
    f32, f16, i16 = mybir.dt.float32, mybir.dt.float16, mybir.dt.int16
    nc = bacc.Bacc("TRN2", num_devices=NC_)

    NLB = NBLK * BS
    WNB = max(NLB, NROW * P)
    xT = nc.dram_tensor("xT", [FIN, NROW * P], f32, kind="ExternalInput")
    idx = nc.dram_tensor("idx", [P, tt * 8], i16, kind="ExternalInput")
    dstrel = nc.dram_tensor("dstrel", [P, tt], f16, kind="ExternalInput")
    iota = nc.dram_tensor("iota", [P, nt_max * P], f16, kind="ExternalInput")
    dinv_col1 = nc.dram_tensor("dinv_col1", [P, NROW], f32, kind="ExternalInput")
    dinv_col2 = nc.dram_tensor("dinv_col2", [P, NBLK], f32, kind="ExternalInput")
    dinv_bc = nc.dram_tensor("dinv_bc", [HID, WNB], f32, kind="ExternalInput")
    W1 = nc.dram_tensor("W1", [FIN, HID], f32, kind="ExternalInput")
    W2 = nc.dram_tensor("W2", [HID, HID], f32, kind="ExternalInput")
    Wc = nc.dram_tensor("Wc", [HID, 1], f32, kind="ExternalInput")
    b1 = nc.dram_tensor("b1", [HID, 1], f32, kind="ExternalInput")
    b2 = nc.dram_tensor("b2", [HID, 1], f32, kind="ExternalInput")
    bc = nc.dram_tensor("bc", [1, 1], f32, kind="ExternalInput")
    out = nc.dram_tensor("out", [1, NLB], f32, kind="ExternalOutput")

    relu = mybir.ActivationFunctionType.Relu
    copy_ = mybir.ActivationFunctionType.Copy

    with tile.TileContext(nc) as tc:
        with (
            tc.tile_pool(name="cst", bufs=1) as cst,
            tc.tile_pool(name="io", bufs=4) as io,
            tc.tile_pool(name="dv", bufs=4) as dv,
            tc.tile_pool(name="msgp", bufs=5) as msgp,
            tc.tile_pool(name="sp", bufs=4) as sp,
            tc.tile_pool(name="work", bufs=4) as work,
            tc.tile_pool(name="agg", bufs=3, space="PSUM") as aggp,
            tc.tile_pool(name="ph", bufs=2, space="PSUM") as php,
            tc.tile_pool(name="pht", bufs=2, space="PSUM") as phtp,
            tc.tile_pool(name="pc", bufs=1, space="PSUM") as pcp,
            tc.tile_pool(name="dram", bufs=1, space="DRAM") as dram,
        ):
            # constants
            W1sb = cst.tile([FIN, HID], f32)
            nc.sync.dma_start(W1sb[:], W1[:])
            W2sb = cst.tile([HID, HID], f32)
            nc.sync.dma_start(W2sb[:], W2[:])
            Wcsb = cst.tile([HID, 1], f32)
            nc.sync.dma_start(Wcsb[:], Wc[:])
            b1sb = cst.tile([HID, 1], f32)
            nc.sync.dma_start(b1sb[:], b1[:])
            b2sb = cst.tile([HID, 1], f32)
            nc.sync.dma_start(b2sb[:], b2[:])
            bcsb = cst.tile([1, 1], f32)
            nc.sync.dma_start(bcsb[:], bc[:])
            dcol1 = cst.tile([P, NROW], f32)
            nc.sync.dma_start(dcol1[:], dinv_col1[:])
            dcol2 = cst.tile([P, NBLK], f32)
            nc.sync.dma_start(dcol2[:], dinv_col2[:])
            iota_sb = cst.tile([P, nt_max * P], f16)
            nc.sync.dma_start(iota_sb[:], iota[:])
            dst_sb = cst.tile([P, tt], f16)
            nc.sync.dma_start(dst_sb[:], dstrel[:])
            # h1T kept fp16; W2 in fp16 so layer-2 matmul is fp16
            h1T = cst.tile([HID, NLB], f16)
            W2h = cst.tile([HID, HID], f16)
            nc.vector.tensor_copy(out=W2h[:], in_=W2sb[:])

            hs1s = dram.tile([NLOC, FP], f16)
            hs2s = dram.tile([NLOC, FP], f16)
            hs1f = [dram.tile([CHUNK, FP], f16, addr_space="Shared",
                              name=f"hs1f{c}") for c in range(NCHUNK)]
            hs2f = [dram.tile([CHUNK, FP], f16, addr_space="Shared",
                              name=f"hs2f{c}") for c in range(NCHUNK)]
            # feature-major copies of the core's own hs (for the epilogue
            # self-loop add); zero-padded to NLB columns
            hsT1 = dram.tile([HID, WNB], f16)
            hsT2 = dram.tile([HID, WNB], f16)

            # ---- phase 1: hs1 = dinv * (x @ W1), fp16 row-padded
            for r in range(NROW):
                w = min(P, NLOC - r * P)
                xb = io.tile([FIN, P], f32, name="xb")
                nc.sync.dma_start(xb[:], xT[:, r * P:(r + 1) * P])
                phh = php.tile([P, HID], f32, name="phh")
                nc.tensor.matmul(out=phh[:], lhsT=xb[:], rhs=W1sb[:],
                                 start=True, stop=True)
                hsb = work.tile([P, FP], f16, name="hsb")
                nc.vector.memset(hsb[:, HID:], 0.0)
                nc.scalar.activation(out=hsb[:, :HID], in_=phh[:], func=copy_,
                                     scale=dcol1[:, r:r + 1])
                nc.sync.dma_start(hs1s[r * P:r * P + w, :], hsb[:w, :])
                # feature-major copy for the self-loop add
                pht_t = phtp.tile([HID, P], f32, name="pht")
                nc.tensor.matmul(out=pht_t[:], lhsT=W1sb[:], rhs=xb[:],
                                 start=True, stop=True)
                dvb1 = dv.tile([HID, P], f32, name="dvb")
                nc.sync.dma_start(dvb1[:], dinv_bc[:, r * P:(r + 1) * P])
                hstb = work.tile([HID, P], f16, name="hstb")
                nc.vector.tensor_tensor(out=hstb[:], in0=pht_t[:],
                                        in1=dvb1[:],
                                        op=mybir.AluOpType.mult)
                nc.sync.dma_start(hsT1[:, r * P:(r + 1) * P], hstb[:])

            for c in range(NCHUNK):
                nc.gpsimd.collective_compute(
                    "AllGather", mybir.AluOpType.bypass,
                    replica_groups=[list(range(NC_))],
                    ins=[hs1s[c * QS:(c + 1) * QS, :]],
                    outs=[hs1f[c][:]],
                )

            # ---- aggregation layers
            for L, table in enumerate([hs1f, hs2f]):
                tile_off = 0   # global tile column offset into dst_sb
                idx_off = 0    # global idx column offset
                for sb in range(NSB):
                    blk_lo = sb * SBB
                    blk_hi = min((sb + 1) * SBB, NBLK)
                    msgs, Ss = [], []
                    for c in range(NCHUNK):
                        st = int(seg_tiles[sb][c])
                        n_idx = st * P
                        ix = io.tile([P, n_idx // 16], i16, name="ix")
                        nc.sync.dma_start(
                            ix[:], idx[:, idx_off:idx_off + n_idx // 16])
                        mg = msgp.tile([P, st, FP], f16, name="mg")
                        if not SKIP_GATHER:
                            nc.gpsimd.dma_gather(
                                mg[:], table[c][:],
                                ix[:], n_idx, n_idx, FP, single_packet=False)
                        else:
                            nc.vector.memset(mg[:, :1, :], 0.0)
                        St = sp.tile([P, st, P], f16, name="St")
                        if SKIP_S:
                            nc.vector.memset(St[:, :1, :], 0.0)
                        else:
                            nc.vector.tensor_tensor(
                                out=St[:],
                                in0=dst_sb[:, tile_off:tile_off + st, None]
                                    .to_broadcast([P, st, P]),
                                in1=iota_sb[:, :st * P]
                                    .rearrange("p (t j) -> p t j", j=P),
                                op=mybir.AluOpType.is_equal)
                        msgs.append(mg)
                        Ss.append(St)
                        tile_off += st
                        idx_off += n_idx // 16

                    for b in range(blk_lo, blk_hi):
                        w = min(BS, NLOC - b * BS)
                        pt = aggp.tile([P, P], f32, name="pt")
                        mms = []
                        for c in range(NCHUNK):
                            off = int(nt[blk_lo:b, c].sum())
                            for t in range(int(nt[b, c])):
                                mms.append((c, off + t))
                        if SKIP_MM:
                            mms = mms[:1]
                        for j, (c, t) in enumerate(mms):
                            nc.tensor.matmul(
                                out=pt[:], lhsT=msgs[c][:, t, :],
                                rhs=Ss[c][:, t, :],
                                start=(j == 0), stop=(j == len(mms) - 1))
                        # epilogue: add self-loop term, scale by dinv[dst]
                        dvb = dv.tile([HID, BS], f32, name="dvb")
                        nc.sync.dma_start(dvb[:],
                                          dinv_bc[:, b * BS:b * BS + BS])
                        sf = dv.tile([HID, BS], f16, name="sf")
                        hsTL = hsT1 if L == 0 else hsT2
                        nc.sync.dma_start(sf[:],
                                          hsTL[:, b * BS:b * BS + BS])
                        t1a = work.tile([HID, BS], f32, name="t1a")
                        nc.vector.tensor_tensor(out=t1a[:], in0=pt[:HID, :BS],
                                                in1=sf[:],
                                                op=mybir.AluOpType.add)
                        t1 = work.tile([HID, BS], f32, name="t1")
                        nc.vector.tensor_tensor(out=t1[:], in0=t1a[:],
                                                in1=dvb[:],
                                                op=mybir.AluOpType.mult)
                        if L == 0:
                            h1s = h1T[:, b * BS:(b + 1) * BS]
                            nc.scalar.activation(
                                out=h1s, in_=t1[:],
                                func=relu, bias=b1sb[:, :1])
                            ph2 = php.tile([P, HID], f32, name="phh")
                            nc.tensor.matmul(
                                out=ph2[:w, :], lhsT=h1s[:, :w],
                                rhs=W2h[:], start=True, stop=True)
                            h2sb = work.tile([P, FP], f16, name="hsb")
                            nc.vector.memset(h2sb[:, HID:], 0.0)
                            nc.scalar.activation(
                                out=h2sb[:w, :HID], in_=ph2[:w, :],
                                func=copy_, scale=dcol2[:w, b:b + 1])
                            nc.sync.dma_start(
                                hs2s[b * BS:b * BS + w, :], h2sb[:w, :])
                            # feature-major hs2 for layer-2 self-loop add
                            ph3 = phtp.tile([HID, BS], f32, name="pht")
                            nc.tensor.matmul(out=ph3[:], lhsT=W2h[:],
                                             rhs=h1s, start=True, stop=True)
                            hst2 = work.tile([HID, BS], f16, name="hstb")
                            nc.vector.tensor_tensor(
                                out=hst2[:], in0=ph3[:], in1=dvb[:],
                                op=mybir.AluOpType.mult)
                            nc.sync.dma_start(
                                hsT2[:, b * BS:(b + 1) * BS], hst2[:])
                        else:
                            h2t = work.tile([HID, BS], f32, name="h2t")
                            nc.scalar.activation(out=h2t[:], in_=t1[:],
                                                 func=relu, bias=b2sb[:, :1])
                            pcl = pcp.tile([1, BS], f32, name="pcl")
                            nc.tensor.matmul(out=pcl[:], lhsT=Wcsb[:],
                                             rhs=h2t[:], start=True, stop=True)
                            oc = work.tile([1, BS], f32, name="oc")
                            nc.vector.tensor_scalar(
                                out=oc[:], in0=pcl[:1, :],
                                scalar1=bcsb[:1, :1],
                                scalar2=None, op0=mybir.AluOpType.add)
                            nc.sync.dma_start(
                                out[:1, b * BS:(b + 1) * BS], oc[:])

                if L == 0:
                    for c in range(NCHUNK):
                        nc.gpsimd.collective_compute(
                            "AllGather", mybir.AluOpType.bypass,
                            replica_groups=[list(range(NC_))],
                            ins=[hs2s[c * QS:(c + 1) * QS, :]],
                            outs=[hs2f[c][:]],
                        )

    nc.compile()
    return nc


_CACHE = {}


def kernel(x, edge_index, W1, b1, W2, b2, Wc, bc):
    x = np.asarray(x, np.float32)
    edge_index = np.asarray(edge_index, np.int32)
    in_maps, nt, seg_tiles, nt_max, tt = _prep(x, edge_index)

    key = (nt_max, tt, nt.tobytes())
    if key not in _CACHE:
        _CACHE[key] = _build(nt, seg_tiles, nt_max, tt)
    nc = _CACHE[key]

    shared = {
        "W1": np.asarray(W1, np.float32),
        "W2": np.asarray(W2, np.float32),
        "Wc": np.asarray(Wc, np.float32).reshape(HID, 1),
        "b1": np.asarray(b1, np.float32).reshape(HID, 1),
        "b2": np.asarray(b2, np.float32).reshape(HID, 1),
        "bc": np.asarray(bc, np.float32).reshape(1, 1),
    }
    for m in in_maps:
        m.update(shared)

    res = run_bass_kernel_spmd(nc, in_maps, core_ids=list(range(NC_)))
    # node j of core k sits at column j (blocks are contiguous BS ranges)
    return np.concatenate(
        [res.results[k]["out"][0, :NLOC] for k in range(NC_)]
    ).astype(np.float32)



# revision 6
# speedup vs baseline: 1.7263x; 1.7263x over previous
"""GCN 2-layer + linear head on 8 Trainium2 NeuronCores (Bass/Tile).

v2 strategy:
- Phase A is REPLICATED: every core computes hs1 = dinv*(x@W1) for ALL
  100k nodes from a shared fp16 x^T input, so layer-1 needs no
  collective at all (the old version did 4 AllGathers per layer).
- Tables are PACKED 128-byte rows ([N, 64] fp16). dma_gather requires
  256B-multiple elements, so gathers fetch PAIRS of rows (idx = src//2)
  and edge tiles are sorted parity-pure so each tile's matmul reads the
  correct 64-feature half of the pair.
- Self-loops ride in the edge stream as ordinary edges, which keeps one
  global table layout for both layers and makes the epilogue a pure
  scale+relu.
- Aggregation is node-major: lhsT = one-hot S [128e, 112d], rhs =
  msg[128e, 64f] -> psum [112d, 64f]; 64-column matmuls.
- ONE AllGather [100000, 64] fp16 between the layers (cost-model:
  15us + 12.8MB @ ~54GB/s = 251us, vs 8 x 175us before).

SPMD: all 8 cores run one identical program; per-core differences live
only in input data (edge indices, dstrel, dinv columns). Tile counts are
padded to the max across cores.
"""

import numpy as np

import concourse.bacc as bacc
import concourse.mybir as mybir
import concourse.tile as tile
from concourse.bass_utils import run_bass_kernel_spmd

# problem shapes (hardcoded per contract)
N = 100000
E = 1600000
FIN = 128
HID = 64

NC_ = 8
P = 128
NLOC = N // NC_            # 12500 nodes per core
NCHUNK = 2                 # 50000-row chunks; pair idx < 25000 fits int16
CHUNK = N // NCHUNK
BS = 112                   # dst nodes per aggregation block
NBLK = (NLOC + BS - 1) // BS   # 112
SBB = 4                    # blocks per superblock (gather/S granularity)
NSB = (NBLK + SBB - 1) // SBB  # 28
GR = 8                     # node-tiles per phase-A group
NG = (N + GR * P - 1) // (GR * P)  # 98 groups -> 100352 padded nodes
NPAD = NG * GR * P
NLB = NBLK * BS            # 12544
SENT = 200.0               # dstrel sentinel for padded edge rows


# ----------------------------------------------------------------- host prep
def _prep(x, edge_index):
    x = np.asarray(x, np.float32)
    src_g = np.asarray(edge_index[0], np.int64)
    dst_g = np.asarray(edge_index[1], np.int64)

    deg = np.bincount(dst_g, minlength=N).astype(np.float32) + 1.0
    dinv = (1.0 / np.sqrt(deg)).astype(np.float32)

    # shared phase-A input: x^T fp16, padded to NPAD columns
    xT = np.zeros((FIN, NPAD), np.float16)
    xT[:, :N] = x.T
    dpad = np.zeros(NPAD, np.float32)
    dpad[:N] = dinv
    dinvA = dpad.reshape(-1, P).T.copy()

    # per-core edge streams -------------------------------------------------
    loop = np.arange(N, dtype=np.int64)
    cores_raw = []
    # cells: (block, chunk, parity) -> per-core counts
    ncell = NBLK * NCHUNK * 2
    counts_all = np.zeros((NC_, ncell), np.int64)
    for k in range(NC_):
        m = (dst_g // NLOC) == k
        s = np.concatenate([src_g[m], loop[k * NLOC:(k + 1) * NLOC]])
        d = np.concatenate([dst_g[m], loop[k * NLOC:(k + 1) * NLOC]]) - k * NLOC
        b = d // BS
        c = s // CHUNK
        par = s % 2
        il = ((s % CHUNK) // 2).astype(np.int16)   # pair index in chunk
        key = (b * NCHUNK + c) * 2 + par
        o = np.argsort(key, kind="stable")
        counts_all[k] = np.bincount(key, minlength=ncell)
        cores_raw.append((il[o], (d % BS)[o],
                          np.concatenate([[0], np.cumsum(counts_all[k])])))

    nt = ((counts_all.max(axis=0) + P - 1) // P).astype(np.int64)
    nt = np.maximum(nt, 1).reshape(NBLK, NCHUNK, 2)

    # tiles per (superblock, chunk) gather call
    seg_tiles = np.array([[nt[sb * SBB:min((sb + 1) * SBB, NBLK), c, :].sum()
                           for c in range(NCHUNK)] for sb in range(NSB)])
    st_max = int(seg_tiles.max())
    tt = int(seg_tiles.sum())          # total tiles per layer

    # matmul schedule per block: list of (chunk, tile-in-(sb,c), parity)
    mms_all = []
    for b in range(NBLK):
        sb = b // SBB
        blk_lo = sb * SBB
        mms = []
        for c in range(NCHUNK):
            off = int(nt[blk_lo:b, c, :].sum())
            for par in range(2):
                for t in range(int(nt[b, c, par])):
                    mms.append((c, off + t, par))
                off += int(nt[b, c, par])
        mms_all.append(mms)

    in_maps = []
    for k in range(NC_):
        il_s, dr_s, cum = cores_raw[k]
        idx_cols, dst_cols = [], []
        for sb in range(NSB):
            blk_lo, blk_hi = sb * SBB, min((sb + 1) * SBB, NBLK)
            for c in range(NCHUNK):
                ils, drs = [], []
                for b in range(blk_lo, blk_hi):
                    for par in range(2):
                        g = (b * NCHUNK + c) * 2 + par
                        a0, a1 = cum[g], cum[g + 1]
                        npad = int(nt[b, c, par] * P - (a1 - a0))
                        ils.append(il_s[a0:a1])
                        ils.append(np.zeros(npad, np.int16))
                        drs.append(dr_s[a0:a1])
                        drs.append(np.full(npad, SENT, np.int64))
                seg_il = np.concatenate(ils)
                seg_dr = np.concatenate(drs)
                wrapped = seg_il.reshape(-1, 16).T          # [16, n/16]
                idx_cols.append(np.tile(wrapped, (8, 1)))   # [128, n/16]
                dst_cols.append(seg_dr.astype(np.float16).reshape(-1, P).T)

        dinvB = np.zeros((P, NBLK), np.float32)
        dloc = dinv[k * NLOC:(k + 1) * NLOC]
        for b in range(NBLK):
            w = min(BS, NLOC - b * BS)
            dinvB[:w, b] = dloc[b * BS:b * BS + w]

        in_maps.append({
            "xT": xT,
            "idx": np.concatenate(idx_cols, axis=1),        # [128, tt*8] i16
            "dstrel": np.concatenate(dst_cols, axis=1),     # [128, tt] fp16
            "dinvA": dinvA,
            "dinvB": dinvB,
        })

    iota_w = np.tile(np.arange(BS, dtype=np.float16), (P, st_max))
    eye = np.eye(P, dtype=np.float16)
    for m in in_maps:
        m["iota"] = iota_w
        m["eye"] = eye
    return in_maps, nt, seg_tiles, st_max, tt, mms_all


# ------------------------------------------------------------- device build
def _build(seg_tiles, st_max, tt, mms_all, has_b1, has_b2, has_bc):
    f32, f16, i16 = mybir.dt.float32, mybir.dt.float16, mybir.dt.int16
    nc = bacc.Bacc("TRN2", num_devices=NC_)

    xT = nc.dram_tensor("xT", [FIN, NPAD], f16, kind="ExternalInput")
    idx = nc.dram_tensor("idx", [P, tt * 8], i16, kind="ExternalInput")
    dstrel = nc.dram_tensor("dstrel", [P, tt], f16, kind="ExternalInput")
    iota = nc.dram_tensor("iota", [P, st_max * BS], f16, kind="ExternalInput")
    eye = nc.dram_tensor("eye", [P, P], f16, kind="ExternalInput")
    dinvA = nc.dram_tensor("dinvA", [P, NPAD // P], f32, kind="ExternalInput")
    dinvB = nc.dram_tensor("dinvB", [P, NBLK], f32, kind="ExternalInput")
    W1 = nc.dram_tensor("W1", [FIN, HID], f32, kind="ExternalInput")
    W2 = nc.dram_tensor("W2", [HID, HID], f32, kind="ExternalInput")
    WcBC = nc.dram_tensor("WcBC", [P, HID], f16, kind="ExternalInput")
    b1bc = nc.dram_tensor("b1bc", [P, HID], f32, kind="ExternalInput")
    b2bc = nc.dram_tensor("b2bc", [P, HID], f32, kind="ExternalInput")
    bc = nc.dram_tensor("bc", [1, 1], f32, kind="ExternalInput")
    out = nc.dram_tensor("out", [NLB, 1], f32, kind="ExternalOutput")

    relu = mybir.ActivationFunctionType.Relu
    copy_ = mybir.ActivationFunctionType.Copy

    with tile.TileContext(nc) as tc:
        with (
            tc.tile_pool(name="cst", bufs=1) as cst,
            tc.tile_pool(name="io", bufs=4) as io,
            tc.tile_pool(name="msgp", bufs=6) as msgp,
            tc.tile_pool(name="sp", bufs=6) as sp,
            tc.tile_pool(name="work", bufs=6) as work,
            tc.tile_pool(name="psA", bufs=2, space="PSUM") as psA,
            tc.tile_pool(name="agg", bufs=3, space="PSUM") as aggp,
            tc.tile_pool(name="ptr", bufs=1, space="PSUM") as ptrp,
            tc.tile_pool(name="p2", bufs=2, space="PSUM") as p2p,
            tc.tile_pool(name="dram", bufs=1, space="DRAM") as dram,
        ):
            # constants
            W1sb = cst.tile([FIN, HID], f32)
            nc.sync.dma_start(W1sb[:], W1[:])
            W1h = cst.tile([FIN, HID], f16)
            nc.vector.tensor_copy(out=W1h[:], in_=W1sb[:])
            W2sb = cst.tile([HID, HID], f32)
            nc.sync.dma_start(W2sb[:], W2[:])
            W2h = cst.tile([HID, HID], f16)
            nc.vector.tensor_copy(out=W2h[:], in_=W2sb[:])
            Wcb = cst.tile([P, HID], f16)
            nc.sync.dma_start(Wcb[:], WcBC[:])
            eyesb = cst.tile([P, P], f16)
            nc.sync.dma_start(eyesb[:], eye[:])
            iotasb = cst.tile([P, st_max * BS], f16)
            nc.sync.dma_start(iotasb[:], iota[:])
            dst_sb = cst.tile([P, tt], f16)
            nc.sync.dma_start(dst_sb[:], dstrel[:])
            dAsb = cst.tile([P, NPAD // P], f32)
            nc.sync.dma_start(dAsb[:], dinvA[:])
            dBsb = cst.tile([P, NBLK], f32)
            nc.sync.dma_start(dBsb[:], dinvB[:])
            if has_b1:
                b1sb = cst.tile([P, HID], f32)
                nc.sync.dma_start(b1sb[:], b1bc[:])
            if has_b2:
                b2sb = cst.tile([P, HID], f32)
                nc.sync.dma_start(b2sb[:], b2bc[:])
            if has_bc:
                bcsb = cst.tile([1, 1], f32)
                nc.sync.dma_start(bcsb[:], bc[:])

            hs1f = dram.tile([NPAD, HID], f16)
            hs2s = dram.tile([NLB, HID], f16)
            hs2f = dram.tile([N, HID], f16, addr_space="Shared", name="hs2f")

            # ---- phase A: hs1 = dinv * (x @ W1) for ALL nodes, fp16 packed
            for g in range(NG):
                xb = io.tile([FIN, GR * P], f16, name="xb")
                nc.sync.dma_start(xb[:], xT[:, g * GR * P:(g + 1) * GR * P])
                ps = psA.tile([P, GR, HID], f32, name="psA")
                for j in range(GR):
                    nc.tensor.matmul(out=ps[:, j, :],
                                     lhsT=xb[:, j * P:(j + 1) * P],
                                     rhs=W1h[:], start=True, stop=True)
                hsg = work.tile([P, GR, HID], f16, name="hsg")
                nc.vector.tensor_tensor(
                    out=hsg[:],
                    in0=ps[:],
                    in1=dAsb[:, g * GR:(g + 1) * GR, None]
                        .to_broadcast([P, GR, HID]),
                    op=mybir.AluOpType.mult)
                nc.sync.dma_start(
                    hs1f[g * GR * P:(g + 1) * GR * P, :]
                        .rearrange("(j p) f -> p j f", p=P),
                    hsg[:])

            # ---- aggregation layers
            for L, table in enumerate([hs1f, hs2f]):
                tile_off = 0
                idx_off = 0
                for sb in range(NSB):
                    blk_lo, blk_hi = sb * SBB, min((sb + 1) * SBB, NBLK)
                    msgs, Ss = [], []
                    for c in range(NCHUNK):
                        st = int(seg_tiles[sb][c])
                        n_idx = st * P
                        ix = io.tile([P, n_idx // 16], i16, name="ix")
                        nc.sync.dma_start(
                            ix[:], idx[:, idx_off:idx_off + n_idx // 16])
                        mg = msgp.tile([P, st, P], f16, name="mg")
                        nc.gpsimd.dma_gather(
                            mg[:],
                            table[c * CHUNK:(c + 1) * CHUNK, :]
                                .rearrange("(q two) f -> q (two f)", two=2),
                            ix[:], n_idx, n_idx, P, single_packet=False)
                        St = sp.tile([P, st, BS], f16, name="St")
                        nc.vector.tensor_tensor(
                            out=St[:],
                            in0=dst_sb[:, tile_off:tile_off + st, None]
                                .to_broadcast([P, st, BS]),
                            in1=iotasb[:, :st * BS]
                                .rearrange("p (t j) -> p t j", j=BS),
                            op=mybir.AluOpType.is_equal)
                        msgs.append(mg)
                        Ss.append(St)
                        tile_off += st
                        idx_off += n_idx // 16

                    for b in range(blk_lo, blk_hi):
                        mms = mms_all[b]
                        pt = aggp.tile([BS, HID], f32, name="pt")
                        for j, (c, t, par) in enumerate(mms):
                            nc.tensor.matmul(
                                out=pt[:],
                                lhsT=Ss[c][:, t, :],
                                rhs=msgs[c][:, t, par * HID:(par + 1) * HID],
                                start=(j == 0), stop=(j == len(mms) - 1))
                        has_b = has_b1 if L == 0 else has_b2
                        if has_b:
                            tsc = work.tile([BS, HID], f32, name="tsc")
                            nc.scalar.activation(out=tsc[:], in_=pt[:],
                                                 func=copy_,
                                                 scale=dBsb[:BS, b:b + 1])
                            tbb = work.tile([BS, HID], f32, name="tbb")
                            bsb = b1sb if L == 0 else b2sb
                            nc.vector.tensor_tensor(out=tbb[:], in0=tsc[:],
                                                    in1=bsb[:BS, :],
                                                    op=mybir.AluOpType.add)
                            hL = work.tile([BS, HID], f16, name="hL")
                            nc.scalar.activation(out=hL[:], in_=tbb[:],
                                                 func=relu)
                        else:
                            hL = work.tile([BS, HID], f16, name="hL")
                            nc.scalar.activation(out=hL[:], in_=pt[:],
                                                 func=relu,
                                                 scale=dBsb[:BS, b:b + 1])
                        if L == 0:
                            # hs2 = dinv * (h1 @ W2): transpose h1 then matmul
                            ptr = ptrp.tile([HID, BS], f16, name="ptr")
                            nc.tensor.transpose(ptr[:], hL[:], eyesb[:BS, :BS])
                            h1T = work.tile([HID, BS], f16, name="h1T")
                            nc.scalar.activation(out=h1T[:], in_=ptr[:],
                                                 func=copy_)
                            ps2 = p2p.tile([BS, HID], f32, name="ps2")
                            nc.tensor.matmul(out=ps2[:], lhsT=h1T[:],
                                             rhs=W2h[:], start=True, stop=True)
                            h2r = work.tile([BS, HID], f16, name="h2r")
                            nc.scalar.activation(out=h2r[:], in_=ps2[:],
                                                 func=copy_,
                                                 scale=dBsb[:BS, b:b + 1])
                            nc.sync.dma_start(
                                hs2s[b * BS:(b + 1) * BS, :], h2r[:])
                        else:
                            # head: out = relu(h2) @ Wc (+ bc)
                            mwc = work.tile([BS, HID], f32, name="mwc")
                            nc.vector.tensor_tensor(out=mwc[:], in0=hL[:],
                                                    in1=Wcb[:BS, :],
                                                    op=mybir.AluOpType.mult)
                            oc = work.tile([BS, 1], f32, name="oc")
                            nc.vector.tensor_reduce(
                                out=oc[:], in_=mwc[:],
                                axis=mybir.AxisListType.X,
                                op=mybir.AluOpType.add)
                            if has_bc:
                                oc2 = work.tile([BS, 1], f32, name="oc2")
                                nc.vector.tensor_scalar(
                                    out=oc2[:], in0=oc[:],
                                    scalar1=bcsb[:1, :1], scalar2=None,
                                    op0=mybir.AluOpType.add)
                                oc = oc2
                            nc.sync.dma_start(
                                out[b * BS:(b + 1) * BS, :], oc[:])

                if L == 0:
                    nc.gpsimd.collective_compute(
                        "AllGather", mybir.AluOpType.bypass,
                        replica_groups=[list(range(NC_))],
                        ins=[hs2s[0:NLOC, :]],
                        outs=[hs2f[:]],
                    )

    nc.compile()
    return nc


_CACHE = {}


def kernel(x, edge_index, W1, b1, W2, b2, Wc, bc):
    x = np.asarray(x, np.float32)
    edge_index = np.asarray(edge_index, np.int32)
    in_maps, nt, seg_tiles, st_max, tt, mms_all = _prep(x, edge_index)

    b1 = np.asarray(b1, np.float32).reshape(-1)
    b2 = np.asarray(b2, np.float32).reshape(-1)
    bc = np.asarray(bc, np.float32).reshape(-1)
    Wc = np.asarray(Wc, np.float32).reshape(-1)
    has_b1 = bool(np.any(b1 != 0))
    has_b2 = bool(np.any(b2 != 0))
    has_bc = bool(np.any(bc != 0))

    key = (st_max, tt, nt.tobytes(), has_b1, has_b2, has_bc)
    if key not in _CACHE:
        _CACHE[key] = _build(seg_tiles, st_max, tt, mms_all,
                             has_b1, has_b2, has_bc)
    nc = _CACHE[key]

    shared = {
        "W1": np.asarray(W1, np.float32),
        "W2": np.asarray(W2, np.float32),
        "WcBC": np.tile(Wc.astype(np.float16), (P, 1)),
        "b1bc": np.tile(b1, (P, 1)),
        "b2bc": np.tile(b2, (P, 1)),
        "bc": bc.reshape(1, 1),
    }
    for m in in_maps:
        m.update(shared)

    res = run_bass_kernel_spmd(nc, in_maps, core_ids=list(range(NC_)))
    return np.concatenate(
        [res.results[k]["out"][:NLOC, 0] for k in range(NC_)]
    ).astype(np.float32)
